# revision 9
# baseline (speedup 1.0000x reference)
"""Trainium2 Bass kernel for nn_MultiHeadModel (moe_routing).

Reference computation:
    route  = argmax(x @ W_lab + b_lab, -1)            # [N]
    z      = x @ W_enc + b_enc                        # [N, 64]
    heads  = einsum('nd,ids->nis', z, W_clf) + b_clf  # [N, 8, 4]
    out    = (heads * onehot(route)).reshape(N, 32)

The problem is HBM-bound; the kernel minimizes bytes moved per token:

  1. Encoder+classifier compose into one linear map: heads = x @ W_eff
     with W_eff = W_enc @ W_clf_flat (host precompute, O(weights)).
  2. x is shipped as fp16 xh (2 B/elem) plus an fp8-e4m3 low-bits
     correction xl8 = e4m3((x - xh) * 2^11) (1 B/elem) that feeds ONLY
     the routing logits. The logit block is computed at scale 2^11 in
     PSUM (argmax is scale-invariant), so the fp8 lo weights
     V8 = e4m3(W_lab) stay in e4m3's normal range. Routing error drops
     to ~1e-5 relative -> ~3 argmax flips out of 524288 (measured on
     the fixed key-0 inputs), rel-err contribution ~1e-3. Compare
     fp16-only (no correction): 86 flips, rel err 1.8e-2 - too close to
     the 2e-2 gate; and fp16 hi/lo (the old scheme): 0 flips but 2x the
     input traffic.
  3. Sparse output: instead of the dense [N, 32] fp32 result (8 MB/core),
     ship the 4 selected head values (fp16) + the 8-wide one-hot mask
     (fp8) = 24 B/token -> 1 MB/core. The host scatters rows into the
     dense zero-filled output (pure reassembly; every route decision and
     head value comes from the device).

Per-core traffic: 16 MB (xh) + 8 MB (xl8) + 1 MB (out) = 25 MB vs the
old 40 MB; DMA roofline ~70 us at 358 GB/s/core.

Device pipeline per 2048-token macro-tile (32 macros/core):
  - DMA xh [128, 2048] fp16, xl8 [128, 2048] fp8 (d_in on partitions,
    tokens on free axis, G-grouped column order - zero device transposes).
  - PE per 128-token tile: lhsT = xh slice ->
      MM hi: moving [W1s|We1][W2s|We2] bf16 [128, 80] with a 2-fold
        0-step out-AP -> psum row [*, 0:40]: logit cols get bf16-double
        2^11*W_lab, head cols get bf16-double W_eff.
    lhsT = xl8 slice ->
      MM lo: moving V8 = e4m3(W_lab) [128, 8], accumulate (start=False)
        onto psum cols 0:8 (scale 2^11 * W_lab * (x-xh)/2^11 matches).
  - DVE on the full macro's psum [128, 16, 64] (16 rows x 64-col stride
    so 8 rows sit in each of 2 psum banks, no row straddles a bank):
      reduce_max logits -> is_equal one-hot mask (fp8 out) ->
      mask x heads -> fp16 [*, 16, 8, 4] -> 3-step pairwise add tree
      over heads (fp16 2x mode) -> out_sel [*, 16, 4].
  - Stores (ACT ring): out_sel fp16 + out_mask fp8.
"""

import sys

if "/opt/trn_rl_repo" not in sys.path:
    sys.path.insert(0, "/opt/trn_rl_repo")

import numpy as np

N_TOTAL = 524288
N_CORES = 8
N_PER_CORE = N_TOTAL // N_CORES  # 65536
D_IN = 128
Y_DIM = 8
S_DIM = 4
D_ENC = 64
W_COLS = Y_DIM + Y_DIM * S_DIM  # 40
OUT_COLS = Y_DIM * S_DIM  # 32
LO_SCALE = 2048.0  # 2^11: logit-block psum scale; keeps V8 in e4m3 range

G = 16                    # tokens per partition per macro-tile
MACRO = 128 * G           # 2048 tokens per macro-tile
N_MACROS = N_PER_CORE // MACRO  # 32
PSROW = 64                # psum row stride (64 f32) so rows never straddle a bank

_CACHE = {}

# test.py can read this after calling kernel() to get profile info
LAST_RESULTS = None


def _build(with_bias: bool):
    import concourse.bacc as bacc
    import concourse.bass as bass
    import concourse.mybir as mybir
    import concourse.tile as tile

    f32 = mybir.dt.float32
    f16 = mybir.dt.float16
    bf16 = mybir.dt.bfloat16
    f8 = mybir.dt.float8e4
    nc = bacc.Bacc("TRN2", target_bir_lowering=False)

    xh_d = nc.dram_tensor("xh", [D_IN, N_PER_CORE], f16, kind="ExternalInput")
    xl8_d = nc.dram_tensor("xl8", [D_IN, N_PER_CORE], f8, kind="ExternalInput")
    w_d = nc.dram_tensor("w_mov", [D_IN, 2 * W_COLS], bf16, kind="ExternalInput")
    v8_d = nc.dram_tensor("v8", [D_IN, Y_DIM], f8, kind="ExternalInput")
    if with_bias:
        b_d = nc.dram_tensor("b_big", [1, W_COLS], f32, kind="ExternalInput")
    osel_d = nc.dram_tensor(
        "out_sel", [N_PER_CORE, S_DIM], f16, kind="ExternalOutput"
    )
    omask_d = nc.dram_tensor(
        "out_mask", [N_PER_CORE, Y_DIM], f8, kind="ExternalOutput"
    )

    with tile.TileContext(nc) as tc:
        with (
            tc.tile_pool(name="const", bufs=1) as const_pool,
            tc.tile_pool(name="xin", bufs=6) as x_pool,
            tc.tile_pool(name="xlin", bufs=6) as xl_pool,
            tc.tile_pool(name="mid", bufs=2) as mid_pool,
            tc.tile_pool(name="osel", bufs=4) as osel_pool,
            tc.tile_pool(name="omask", bufs=4) as omask_pool,
            tc.tile_pool(name="small", bufs=4) as small_pool,
            tc.tile_pool(name="bigp", bufs=3, space=bass.MemorySpace.PSUM) as bigp_pool,
        ):
            w_sb = const_pool.tile([D_IN, 2 * W_COLS], bf16)
            nc.sync.dma_start(w_sb[:], w_d[:])
            v8_sb = const_pool.tile([D_IN, Y_DIM], f8)
            nc.sync.dma_start(v8_sb[:], v8_d[:])
            # dummy SWDGE transfer nothing waits on: warms the Q7 DMA ucode
            # (~6 us first-call IRAM load) before the first store needs it
            swdge_warm = const_pool.tile([D_IN, Y_DIM], f8)
            nc.gpsimd.dma_start(swdge_warm[:], v8_d[:])

            if with_bias:
                ones_sb = const_pool.tile([1, 128], f32)
                nc.gpsimd.memset(ones_sb[:], 1.0)
                b_row = const_pool.tile([1, W_COLS], f32)
                nc.sync.dma_start(b_row[:], b_d[:])
                with tc.tile_pool(
                    name="biasp", bufs=1, space=bass.MemorySpace.PSUM
                ) as biasp_pool:
                    bias_ps = biasp_pool.tile([128, W_COLS], f32)
                    nc.tensor.matmul(bias_ps[:], ones_sb[:], b_row[:])
                    bias_sb = const_pool.tile([128, W_COLS], f32)
                    nc.scalar.copy(bias_sb[:], bias_ps[:])

            # 2 macros per DMA batch: 1 MB xh / 512 KB xl8 / 64 KB store
            # transfers (>= 64 KB per SDMA queue, out of the
            # descriptor-dominated regime), with xh on the sync HWDGE
            # ring and xl8 + stores on the ACT ring so the two load
            # streams don't serialize on one FIFO ring.
            for pair in range(N_MACROS // 2):
                r0 = pair * 2 * MACRO
                # alternate ring assignment per pair: each HWDGE ring
                # carries 12.5 MB total (balanced), and a pair's two loads
                # always land on different rings so they stream in parallel
                ring_a = nc.sync if pair % 2 == 0 else nc.scalar
                ring_b = nc.scalar if pair % 2 == 0 else nc.sync
                xh_sb = x_pool.tile([D_IN, 2 * MACRO], f16)
                ring_a.dma_start(xh_sb[:], xh_d[:, r0 : r0 + 2 * MACRO])
                xl8_sb = xl_pool.tile([D_IN, 2 * MACRO], f8)
                ring_b.dma_start(xl8_sb[:], xl8_d[:, r0 : r0 + 2 * MACRO])

                osel_sb = osel_pool.tile([128, 2, G, S_DIM], f16)
                omask_sb = omask_pool.tile([128, 2, G, Y_DIM], f8)

                for h in range(2):
                    c0 = h * MACRO
                    big_ps = bigp_pool.tile([128, G, PSROW], f32)
                    for t in range(G):
                        hs = xh_sb[:, c0 + t * 128 : c0 + (t + 1) * 128]
                        ls = xl8_sb[:, c0 + t * 128 : c0 + (t + 1) * 128]
                        row = big_ps[:, t, 0:W_COLS]
                        row_fold = row[:, None, :].broadcast_to([128, 2, W_COLS])
                        # hi: cols 0:8  = xh @ (W1s + W2s)  (= 2^11 x_hi @ W_lab)
                        #     cols 8:40 = xh @ (We1 + We2)  (= x_hi @ W_eff)
                        nc.tensor.matmul(
                            row_fold,
                            hs,
                            w_sb[:],
                            start=True,
                            stop=False,
                            skip_group_check=True,
                        )
                        # lo: cols 0:8 += xl8 @ e4m3(W_lab) (= 2^11 x_lo @ W_lab)
                        nc.tensor.matmul(
                            big_ps[:, t, 0:Y_DIM],
                            ls,
                            v8_sb[:],
                            start=False,
                            stop=True,
                            skip_group_check=True,
                        )

                    if with_bias:
                        nc.vector.tensor_tensor(
                            big_ps[:, :, 0:W_COLS],
                            big_ps[:, :, 0:W_COLS],
                            bias_sb[:][:, None, :].broadcast_to([128, G, W_COLS]),
                            mybir.AluOpType.add,
                        )

                    maxl = small_pool.tile([128, G], f32)
                    nc.vector.tensor_reduce(
                        maxl[:],
                        big_ps[:, :, 0:Y_DIM],
                        axis=mybir.AxisListType.X,
                        op=mybir.AluOpType.max,
                    )
                    nc.vector.tensor_tensor(
                        omask_sb[:, h],
                        big_ps[:, :, 0:Y_DIM],
                        maxl[:][:, :, None].broadcast_to([128, G, Y_DIM]),
                        mybir.AluOpType.is_equal,
                    )
                    masked = mid_pool.tile([128, G, Y_DIM, S_DIM], f16)
                    nc.vector.tensor_tensor(
                        masked[:],
                        big_ps[:, :, Y_DIM:W_COLS].rearrange(
                            "p g (i s) -> p g i s", s=S_DIM
                        ),
                        omask_sb[:, h][:, :, :, None].broadcast_to(
                            [128, G, Y_DIM, S_DIM]
                        ),
                        mybir.AluOpType.mult,
                    )
                    t1 = mid_pool.tile([128, G, 4, S_DIM], f16)
                    nc.vector.tensor_tensor(
                        t1[:],
                        masked[:, :, 0:4, :],
                        masked[:, :, 4:8, :],
                        mybir.AluOpType.add,
                    )
                    t2 = mid_pool.tile([128, G, 2, S_DIM], f16)
                    nc.vector.tensor_tensor(
                        t2[:],
                        t1[:, :, 0:2, :],
                        t1[:, :, 2:4, :],
                        mybir.AluOpType.add,
                    )
                    nc.vector.tensor_tensor(
                        osel_sb[:, h],
                        t2[:, :, 0, :],
                        t2[:, :, 1, :],
                        mybir.AluOpType.add,
                    )

                # stores go out on the SWDGE (gpsimd) ring: they wait on
                # DVE completion, and on a HWDGE FIFO ring that wait would
                # head-of-line-block the next pair's loads.
                nc.gpsimd.dma_start(
                    osel_d[r0 : r0 + 2 * MACRO, :].rearrange(
                        "(m p g) j -> p m g j", m=2, p=128
                    ),
                    osel_sb[:],
                )
                nc.gpsimd.dma_start(
                    omask_d[r0 : r0 + 2 * MACRO, :].rearrange(
                        "(m p g) j -> p m g j", m=2, p=128
                    ),
                    omask_sb[:],
                )

    nc.compile()
    return nc


def _get_nc(with_bias: bool):
    key = ("nc", with_bias)
    if key not in _CACHE:
        _CACHE[key] = _build(with_bias)
    return _CACHE[key]


def _host_transpose_shard(xs):
    """[65536, d] -> [d, 65536] with G-grouped column order.

    Device column (m, t*128 + p) must hold token m*MACRO + p*G + t so that
    the PSUM/output partition p covers G consecutive tokens per macro.
    """
    d = xs.shape[1]
    xs4 = xs.reshape(N_MACROS, 128, G, d)  # [m, p, t, d]
    return np.ascontiguousarray(
        xs4.transpose(3, 0, 2, 1).reshape(d, N_PER_CORE)
    )


def kernel(x, W_lab, b_lab, W_enc, b_enc, W_clf, b_clf):
    global LAST_RESULTS
    from concourse.bass_utils import run_bass_kernel_spmd

    x = np.asarray(x, dtype=np.float32)
    W_lab = np.asarray(W_lab, dtype=np.float32)
    b_lab = np.asarray(b_lab, dtype=np.float32)
    W_enc = np.asarray(W_enc, dtype=np.float32)
    b_enc = np.asarray(b_enc, dtype=np.float32)
    W_clf = np.asarray(W_clf, dtype=np.float32)
    b_clf = np.asarray(b_clf, dtype=np.float32)

    # Fold encoder + classifier into one [128, 32] map (all linear).
    w_clf_flat = np.transpose(W_clf, (1, 0, 2)).reshape(D_ENC, OUT_COLS)
    w_eff = (W_enc.astype(np.float64) @ w_clf_flat.astype(np.float64)).astype(
        np.float32
    )
    b_eff = (
        b_enc.astype(np.float64) @ w_clf_flat.astype(np.float64)
        + b_clf.reshape(OUT_COLS).astype(np.float64)
    ).astype(np.float32)
    b_big = np.concatenate([b_lab * LO_SCALE, b_eff]).astype(np.float32)  # [40]

    import ml_dtypes

    bf = ml_dtypes.bfloat16
    f8 = ml_dtypes.float8_e4m3

    xh = x.astype(np.float16)
    xl8 = ((x - xh.astype(np.float32)) * LO_SCALE).astype(f8)

    def bf16_double(w):
        w1 = w.astype(bf)
        w2 = (w - w1.astype(np.float32)).astype(bf)
        return w1, w2

    w1, w2 = bf16_double(W_lab * LO_SCALE)
    we1, we2 = bf16_double(w_eff)
    w_mov = np.ascontiguousarray(
        np.concatenate([w1, we1, w2, we2], axis=1).astype(bf)
    )  # [128, 80] bf16
    v8 = np.ascontiguousarray(W_lab.astype(f8))  # [128, 8] fp8

    with_bias = bool(np.any(b_big != 0.0))
    nc = _get_nc(with_bias)

    in_maps = []
    for i in range(N_CORES):
        sl = slice(i * N_PER_CORE, (i + 1) * N_PER_CORE)
        m = {
            "xh": _host_transpose_shard(xh[sl]),
            "xl8": _host_transpose_shard(xl8[sl]),
            "w_mov": w_mov,
            "v8": v8,
        }
        if with_bias:
            m["b_big"] = b_big.reshape(1, W_COLS)
        in_maps.append(m)

    res = run_bass_kernel_spmd(nc, in_maps, list(range(N_CORES)))
    LAST_RESULTS = res

    sel = np.concatenate(
        [np.asarray(res.results[i]["out_sel"]) for i in range(N_CORES)], axis=0
    ).astype(np.float32)  # [N, 4]
    mask_u8 = np.concatenate(
        [
            np.asarray(res.results[i]["out_mask"]).view(np.uint8)
            for i in range(N_CORES)
        ],
        axis=0,
    )  # [N, 8] raw fp8 bytes; 1.0 -> 0x38, 0.0 -> 0
    route = np.argmax(mask_u8, axis=1)

    out = np.zeros((N_TOTAL, OUT_COLS), dtype=np.float32)
    cols = route[:, None] * S_DIM + np.arange(S_DIM)[None, :]
    np.put_along_axis(out, cols, sel, axis=1)
    return out


# revision 11
# speedup vs baseline: 1.0263x; 1.0263x over previous
"""Trainium2 Bass kernel for nn_MultiHeadModel (moe_routing).

Reference computation:
    route  = argmax(x @ W_lab + b_lab, -1)            # [N]
    z      = x @ W_enc + b_enc                        # [N, 64]
    heads  = einsum('nd,ids->nis', z, W_clf) + b_clf  # [N, 8, 4]
    out    = (heads * onehot(route)).reshape(N, 32)

The problem is HBM-bound; the kernel minimizes bytes moved per token:

  1. Encoder+classifier compose into one linear map: heads = x @ W_eff
     with W_eff = W_enc @ W_clf_flat (host precompute, O(weights)).
  2. x is shipped as fp16 xh (2 B/elem) plus an fp8-e4m3 low-bits
     correction xl8 = e4m3((x - xh) * 2^11) (1 B/elem) that feeds ONLY
     the routing logits. The logit block is computed at scale 2^11 in
     PSUM (argmax is scale-invariant), so the fp8 lo weights
     V8 = e4m3(W_lab) stay in e4m3's normal range. Routing error drops
     to ~1e-5 relative -> ~3 argmax flips out of 524288 (measured on
     the fixed key-0 inputs), rel-err contribution ~1e-3. Compare
     fp16-only (no correction): 86 flips, rel err 1.8e-2 - too close to
     the 2e-2 gate; and fp16 hi/lo (the old scheme): 0 flips but 2x the
     input traffic.
  3. Sparse output: instead of the dense [N, 32] fp32 result (8 MB/core),
     ship the 4 selected head values (fp16) + the 8-wide one-hot mask
     (fp8) = 24 B/token -> 1 MB/core. The host scatters rows into the
     dense zero-filled output (pure reassembly; every route decision and
     head value comes from the device).

Per-core traffic: 16 MB (xh) + 8 MB (xl8) + 1 MB (out) = 25 MB vs the
old 40 MB; DMA roofline ~70 us at 358 GB/s/core.

Device pipeline per 2048-token macro-tile (32 macros/core):
  - DMA xh [128, 2048] fp16, xl8 [128, 2048] fp8 (d_in on partitions,
    tokens on free axis, G-grouped column order - zero device transposes).
  - PE per 128-token tile: lhsT = xh slice ->
      MM hi: moving [W1s|We1][W2s|We2] bf16 [128, 80] with a 2-fold
        0-step out-AP -> psum row [*, 0:40]: logit cols get bf16-double
        2^11*W_lab, head cols get bf16-double W_eff.
    lhsT = xl8 slice ->
      MM lo: moving V8 = e4m3(W_lab) [128, 8], accumulate (start=False)
        onto psum cols 0:8 (scale 2^11 * W_lab * (x-xh)/2^11 matches).
  - DVE on the full macro's psum [128, 16, 64] (16 rows x 64-col stride
    so 8 rows sit in each of 2 psum banks, no row straddles a bank):
      reduce_max logits -> is_equal one-hot mask (fp8 out) ->
      mask x heads -> fp16 [*, 16, 8, 4] -> 3-step pairwise add tree
      over heads (fp16 2x mode) -> out_sel [*, 16, 4].
  - Stores (ACT ring): out_sel fp16 + out_mask fp8.
"""

import sys

if "/opt/trn_rl_repo" not in sys.path:
    sys.path.insert(0, "/opt/trn_rl_repo")

import numpy as np

N_TOTAL = 524288
N_CORES = 8
N_PER_CORE = N_TOTAL // N_CORES  # 65536
D_IN = 128
Y_DIM = 8
S_DIM = 4
D_ENC = 64
W_COLS = Y_DIM + Y_DIM * S_DIM  # 40
OUT_COLS = Y_DIM * S_DIM  # 32
LO_SCALE = 2048.0  # 2^11: logit-block psum scale; keeps V8 in e4m3 range

G = 16                    # tokens per partition per macro-tile
MACRO = 128 * G           # 2048 tokens per macro-tile
N_MACROS = N_PER_CORE // MACRO  # 32
PSROW = 64                # psum row stride (64 f32) so rows never straddle a bank

_CACHE = {}

# test.py can read this after calling kernel() to get profile info
LAST_RESULTS = None


def _build(with_bias: bool):
    import concourse.bacc as bacc
    import concourse.bass as bass
    import concourse.mybir as mybir
    import concourse.tile as tile

    f32 = mybir.dt.float32
    f16 = mybir.dt.float16
    bf16 = mybir.dt.bfloat16
    f8 = mybir.dt.float8e4
    nc = bacc.Bacc("TRN2", target_bir_lowering=False)

    xh_d = nc.dram_tensor("xh", [D_IN, N_PER_CORE], f16, kind="ExternalInput")
    xl8_d = nc.dram_tensor("xl8", [D_IN, N_PER_CORE], f8, kind="ExternalInput")
    w_d = nc.dram_tensor("w_mov", [D_IN, 2 * W_COLS], bf16, kind="ExternalInput")
    v8_d = nc.dram_tensor("v8", [D_IN, Y_DIM], f8, kind="ExternalInput")
    if with_bias:
        b_d = nc.dram_tensor("b_big", [1, W_COLS], f32, kind="ExternalInput")
    osel_d = nc.dram_tensor(
        "out_sel", [N_PER_CORE, S_DIM], f16, kind="ExternalOutput"
    )
    omask_d = nc.dram_tensor(
        "out_mask", [N_PER_CORE, Y_DIM], f8, kind="ExternalOutput"
    )

    with tile.TileContext(nc) as tc:
        with (
            tc.tile_pool(name="const", bufs=1) as const_pool,
            tc.tile_pool(name="xin", bufs=6) as x_pool,
            tc.tile_pool(name="xlin", bufs=6) as xl_pool,
            tc.tile_pool(name="mid", bufs=2) as mid_pool,
            tc.tile_pool(name="osel", bufs=4) as osel_pool,
            tc.tile_pool(name="omask", bufs=4) as omask_pool,
            tc.tile_pool(name="small", bufs=4) as small_pool,
            tc.tile_pool(name="bigp", bufs=2, space=bass.MemorySpace.PSUM) as bigp_pool,
        ):
            w_sb = const_pool.tile([D_IN, 2 * W_COLS], bf16)
            nc.sync.dma_start(w_sb[:], w_d[:])
            v8_sb = const_pool.tile([D_IN, Y_DIM], f8)
            nc.sync.dma_start(v8_sb[:], v8_d[:])
            # dummy SWDGE transfer nothing waits on: warms the Q7 DMA ucode
            # (~6 us first-call IRAM load) before the first store needs it
            swdge_warm = const_pool.tile([D_IN, Y_DIM], f8)
            nc.gpsimd.dma_start(swdge_warm[:], v8_d[:])

            if with_bias:
                ones_sb = const_pool.tile([1, 128], f32)
                nc.gpsimd.memset(ones_sb[:], 1.0)
                b_row = const_pool.tile([1, W_COLS], f32)
                nc.sync.dma_start(b_row[:], b_d[:])
                with tc.tile_pool(
                    name="biasp", bufs=1, space=bass.MemorySpace.PSUM
                ) as biasp_pool:
                    bias_ps = biasp_pool.tile([128, W_COLS], f32)
                    nc.tensor.matmul(bias_ps[:], ones_sb[:], b_row[:])
                    bias_sb = const_pool.tile([128, W_COLS], f32)
                    nc.scalar.copy(bias_sb[:], bias_ps[:])

            # 2 macros per DMA batch: 1 MB xh / 512 KB xl8 / 64 KB store
            # transfers (>= 64 KB per SDMA queue, out of the
            # descriptor-dominated regime), with xh on the sync HWDGE
            # ring and xl8 + stores on the ACT ring so the two load
            # streams don't serialize on one FIFO ring.
            for pair in range(N_MACROS // 2):
                r0 = pair * 2 * MACRO
                # alternate ring assignment per pair: each HWDGE ring
                # carries 12.5 MB total (balanced), and a pair's two loads
                # always land on different rings so they stream in parallel
                ring_a = nc.sync if pair % 2 == 0 else nc.scalar
                ring_b = nc.scalar if pair % 2 == 0 else nc.sync
                xh_sb = x_pool.tile([D_IN, 2 * MACRO], f16)
                ring_a.dma_start(xh_sb[:], xh_d[:, r0 : r0 + 2 * MACRO])
                xl8_sb = xl_pool.tile([D_IN, 2 * MACRO], f8)
                ring_b.dma_start(xl8_sb[:], xl8_d[:, r0 : r0 + 2 * MACRO])

                osel_sb = osel_pool.tile([128, 2 * G, S_DIM], f16)
                omask_sb = omask_pool.tile([128, 2 * G, Y_DIM], f8)

                big_ps = bigp_pool.tile([128, 2 * G, PSROW], f32)
                for t in range(2 * G):
                    hs = xh_sb[:, t * 128 : (t + 1) * 128]
                    ls = xl8_sb[:, t * 128 : (t + 1) * 128]
                    row = big_ps[:, t, 0:W_COLS]
                    row_fold = row[:, None, :].broadcast_to([128, 2, W_COLS])
                    # hi: cols 0:8  = xh @ (W1s + W2s)  (= 2^11 x_hi @ W_lab)
                    #     cols 8:40 = xh @ (We1 + We2)  (= x_hi @ W_eff)
                    nc.tensor.matmul(
                        row_fold,
                        hs,
                        w_sb[:],
                        start=True,
                        stop=False,
                        skip_group_check=True,
                    )
                    # lo: cols 0:8 += xl8 @ e4m3(W_lab) (= 2^11 x_lo @ W_lab)
                    nc.tensor.matmul(
                        big_ps[:, t, 0:Y_DIM],
                        ls,
                        v8_sb[:],
                        start=False,
                        stop=True,
                        skip_group_check=True,
                    )

                if with_bias:
                    nc.vector.tensor_tensor(
                        big_ps[:, :, 0:W_COLS],
                        big_ps[:, :, 0:W_COLS],
                        bias_sb[:][:, None, :].broadcast_to([128, 2 * G, W_COLS]),
                        mybir.AluOpType.add,
                    )

                maxl = small_pool.tile([128, 2 * G], f32)
                nc.vector.tensor_reduce(
                    maxl[:],
                    big_ps[:, :, 0:Y_DIM],
                    axis=mybir.AxisListType.X,
                    op=mybir.AluOpType.max,
                )
                nc.vector.tensor_tensor(
                    omask_sb[:],
                    big_ps[:, :, 0:Y_DIM],
                    maxl[:][:, :, None].broadcast_to([128, 2 * G, Y_DIM]),
                    mybir.AluOpType.is_equal,
                )
                masked = mid_pool.tile([128, 2 * G, Y_DIM, S_DIM], f16)
                nc.vector.tensor_tensor(
                    masked[:],
                    big_ps[:, :, Y_DIM:W_COLS].rearrange(
                        "p g (i s) -> p g i s", s=S_DIM
                    ),
                    omask_sb[:][:, :, :, None].broadcast_to(
                        [128, 2 * G, Y_DIM, S_DIM]
                    ),
                    mybir.AluOpType.mult,
                )
                t1 = mid_pool.tile([128, 2 * G, 4, S_DIM], f16)
                nc.vector.tensor_tensor(
                    t1[:],
                    masked[:, :, 0:4, :],
                    masked[:, :, 4:8, :],
                    mybir.AluOpType.add,
                )
                t2 = mid_pool.tile([128, 2 * G, 2, S_DIM], f16)
                nc.vector.tensor_tensor(
                    t2[:],
                    t1[:, :, 0:2, :],
                    t1[:, :, 2:4, :],
                    mybir.AluOpType.add,
                )
                nc.vector.tensor_tensor(
                    osel_sb[:],
                    t2[:, :, 0, :],
                    t2[:, :, 1, :],
                    mybir.AluOpType.add,
                )

                # stores go out on the SWDGE (gpsimd) ring: they wait on
                # DVE completion, and on a HWDGE FIFO ring that wait would
                # head-of-line-block the next pair's loads.
                nc.gpsimd.dma_start(
                    osel_d[r0 : r0 + 2 * MACRO, :].rearrange(
                        "(m p g) j -> p m g j", m=2, p=128
                    ),
                    osel_sb[:].rearrange("p (m g) j -> p m g j", m=2),
                )
                nc.gpsimd.dma_start(
                    omask_d[r0 : r0 + 2 * MACRO, :].rearrange(
                        "(m p g) j -> p m g j", m=2, p=128
                    ),
                    omask_sb[:].rearrange("p (m g) j -> p m g j", m=2),
                )

    nc.compile()
    return nc


def _get_nc(with_bias: bool):
    key = ("nc", with_bias)
    if key not in _CACHE:
        _CACHE[key] = _build(with_bias)
    return _CACHE[key]


def _host_transpose_shard(xs):
    """[65536, d] -> [d, 65536] with G-grouped column order.

    Device column (m, t*128 + p) must hold token m*MACRO + p*G + t so that
    the PSUM/output partition p covers G consecutive tokens per macro.
    """
    d = xs.shape[1]
    xs4 = xs.reshape(N_MACROS, 128, G, d)  # [m, p, t, d]
    return np.ascontiguousarray(
        xs4.transpose(3, 0, 2, 1).reshape(d, N_PER_CORE)
    )


def kernel(x, W_lab, b_lab, W_enc, b_enc, W_clf, b_clf):
    global LAST_RESULTS
    from concourse.bass_utils import run_bass_kernel_spmd

    x = np.asarray(x, dtype=np.float32)
    W_lab = np.asarray(W_lab, dtype=np.float32)
    b_lab = np.asarray(b_lab, dtype=np.float32)
    W_enc = np.asarray(W_enc, dtype=np.float32)
    b_enc = np.asarray(b_enc, dtype=np.float32)
    W_clf = np.asarray(W_clf, dtype=np.float32)
    b_clf = np.asarray(b_clf, dtype=np.float32)

    # Fold encoder + classifier into one [128, 32] map (all linear).
    w_clf_flat = np.transpose(W_clf, (1, 0, 2)).reshape(D_ENC, OUT_COLS)
    w_eff = (W_enc.astype(np.float64) @ w_clf_flat.astype(np.float64)).astype(
        np.float32
    )
    b_eff = (
        b_enc.astype(np.float64) @ w_clf_flat.astype(np.float64)
        + b_clf.reshape(OUT_COLS).astype(np.float64)
    ).astype(np.float32)
    b_big = np.concatenate([b_lab * LO_SCALE, b_eff]).astype(np.float32)  # [40]

    import ml_dtypes

    bf = ml_dtypes.bfloat16
    f8 = ml_dtypes.float8_e4m3

    xh = x.astype(np.float16)
    xl8 = ((x - xh.astype(np.float32)) * LO_SCALE).astype(f8)

    def bf16_double(w):
        w1 = w.astype(bf)
        w2 = (w - w1.astype(np.float32)).astype(bf)
        return w1, w2

    w1, w2 = bf16_double(W_lab * LO_SCALE)
    we1, we2 = bf16_double(w_eff)
    w_mov = np.ascontiguousarray(
        np.concatenate([w1, we1, w2, we2], axis=1).astype(bf)
    )  # [128, 80] bf16
    v8 = np.ascontiguousarray(W_lab.astype(f8))  # [128, 8] fp8

    with_bias = bool(np.any(b_big != 0.0))
    nc = _get_nc(with_bias)

    in_maps = []
    for i in range(N_CORES):
        sl = slice(i * N_PER_CORE, (i + 1) * N_PER_CORE)
        m = {
            "xh": _host_transpose_shard(xh[sl]),
            "xl8": _host_transpose_shard(xl8[sl]),
            "w_mov": w_mov,
            "v8": v8,
        }
        if with_bias:
            m["b_big"] = b_big.reshape(1, W_COLS)
        in_maps.append(m)

    res = run_bass_kernel_spmd(nc, in_maps, list(range(N_CORES)))
    LAST_RESULTS = res

    sel = np.concatenate(
        [np.asarray(res.results[i]["out_sel"]) for i in range(N_CORES)], axis=0
    ).astype(np.float32)  # [N, 4]
    mask_u8 = np.concatenate(
        [
            np.asarray(res.results[i]["out_mask"]).view(np.uint8)
            for i in range(N_CORES)
        ],
        axis=0,
    )  # [N, 8] raw fp8 bytes; 1.0 -> 0x38, 0.0 -> 0
    route = np.argmax(mask_u8, axis=1)

    out = np.zeros((N_TOTAL, OUT_COLS), dtype=np.float32)
    cols = route[:, None] * S_DIM + np.arange(S_DIM)[None, :]
    np.put_along_axis(out, cols, sel, axis=1)
    return out
